# revision 5
# baseline (speedup 1.0000x reference)
"""DUQ RBF head kernel for Trainium2 (8 NeuronCores, batch-parallel).

Computes out[b,c,h,w] = exp(gamma * mean_e (einsum('bfhw,ecf', x, W) - m/N)^2)
for features [8,512,128,128], weights [16,64,512], m [16,64], N [64].

Strategy: data-parallel over batch (1 image per core). Per core, one big
matmul [ec=1024, f=512] @ [f=512, pix=16384] on the tensor engine.

v4: both matmul operands fp16 (exact products, fp32 PSUM accumulate;
quantization error ~2e-3 << 2e-2 tolerance). fp16 halves feature DMA bytes
and makes LDWEIGHTS fast (FWL), so the matmul stream runs at the 216 ns
issue-gap floor. Pixel super-tiles of 2048 (4 PSUM banks, 2 ping-pong)
with epilogue ACT/DVE ops at free-dim 2048 amortize fixed op overheads;
the last 2048 pixels run as four 512-wide tiles so the post-matmul serial
tail (square+fold+exp+store) is short. Features stream on the Sync HWDGE
queue; the single big-descriptor weight DMA + centroid bias ride the
Scalar HWDGE queue in parallel so the first matmul starts as early as
possible.
"""

import numpy as np

import concourse.bacc as bacc_mod
import concourse.mybir as mybir
import concourse.tile as tile
from concourse.bass_utils import run_bass_kernel_spmd

dt = mybir.dt
Act = mybir.ActivationFunctionType

B, F, H, W = 8, 512, 128, 128
E, C = 16, 64
PIX = H * W           # 16384 pixels per image
ST = 2048             # super-tile (4 psum banks)
NST = 7               # super-tiles; remainder processed as 512-wide tiles
SMALL = 512
NSMALL = (PIX - NST * ST) // SMALL  # 4
MCH = (E * C) // 128  # 8 ec-chunks of 128 partitions
KCH = F // 128        # 4 contraction chunks
LENGTH_SCALE = 0.1
GAMMA = -1.0 / (2.0 * LENGTH_SCALE**2)   # -50.0
EXP_SCALE = GAMMA / E                    # -3.125


def _build():
    nc = bacc_mod.Bacc(None)
    feat_d = nc.declare_dram_parameter("feat", [F, PIX], dt.float16, isOutput=False)
    wt_d = nc.declare_dram_parameter("wt", [F, E * C], dt.float16, isOutput=False)
    negc_d = nc.declare_dram_parameter("negc", [128, MCH], dt.float32, isOutput=False)
    out_d = nc.declare_dram_parameter("out", [C, PIX], dt.float32, isOutput=True)

    feat_k = feat_d.rearrange("(k p) x -> p k x", k=KCH)
    wt_k = wt_d.rearrange("(k p) m -> p k m", k=KCH)

    # (start_px, width) for each pixel tile
    tiles = [(t * ST, ST) for t in range(NST)]
    tiles += [(NST * ST + s * SMALL, SMALL) for s in range(NSMALL)]

    with tile.TileContext(nc) as tc:
        with (
            tc.tile_pool(name="singles", bufs=1) as singles,
            tc.tile_pool(name="xin", bufs=3) as xin,
            tc.tile_pool(name="sqp", bufs=3) as sqp,
            tc.tile_pool(name="accp", bufs=2) as accp,
            tc.tile_pool(name="outp", bufs=2) as outp,
            tc.tile_pool(name="ps", bufs=2, space="PSUM") as ps,
        ):
            # Weights as ONE DMA with 2 KiB descriptors on the Scalar HWDGE
            # ring (separate from the Sync ring carrying features).
            wsall = singles.tile([128, KCH, E * C], dt.float16, tag="wsall")
            nc.scalar.dma_start(out=wsall, in_=wt_k)
            negc_sb = singles.tile([128, MCH], dt.float32, tag="negc")
            nc.scalar.dma_start(out=negc_sb, in_=negc_d[:, :])

            xtiles = []
            for px0, width in tiles:
                xt = []
                for k in range(KCH):
                    xtk = xin.tile([128, width], dt.float16, tag=f"x{k}")
                    nc.sync.dma_start(
                        out=xtk, in_=feat_k[:, k, px0 : px0 + width]
                    )
                    xt.append(xtk)
                xtiles.append(xt)

            for (px0, width), xt in zip(tiles, xtiles):
                nsl = width // 512
                acc = accp.tile([128, width], dt.float32, tag="acc")
                for m in range(MCH):
                    pst = ps.tile([128, width], dt.float32, tag="mm")
                    for k in range(KCH):
                        for s in range(nsl):
                            sl = slice(s * 512, (s + 1) * 512)
                            nc.tensor.matmul(
                                out=pst[:, sl],
                                lhsT=wsall[:, k, m * 128 : (m + 1) * 128],
                                rhs=xt[k][:, sl],
                                start=(k == 0), stop=(k == KCH - 1),
                            )
                    if m == 0:
                        nc.scalar.activation(
                            out=acc, in_=pst, func=Act.Square,
                            bias=negc_sb[:, 0:1], scale=1.0,
                        )
                    else:
                        sq = sqp.tile([128, width], dt.float32, tag="sq")
                        nc.scalar.activation(
                            out=sq, in_=pst, func=Act.Square,
                            bias=negc_sb[:, m : m + 1], scale=1.0,
                        )
                        nc.vector.tensor_add(out=acc, in0=acc, in1=sq)

                tmp = outp.tile([64, width], dt.float32, tag="tmp")
                nc.vector.tensor_copy(out=tmp, in_=acc[64:128, :])
                hc = outp.tile([64, width], dt.float32, tag="hc")
                nc.vector.tensor_add(out=hc, in0=acc[0:64, :], in1=tmp)
                eo = outp.tile([64, width], dt.float32, tag="eo")
                nc.scalar.activation(
                    out=eo, in_=hc, func=Act.Exp, bias=0.0, scale=EXP_SCALE
                )
                nc.scalar.dma_start(out=out_d[:, px0 : px0 + width], in_=eo)

    nc.finalize()
    return nc


_NC_CACHE = {}


def _get_nc():
    if "nc" not in _NC_CACHE:
        _NC_CACHE["nc"] = _build()
    return _NC_CACHE["nc"]


def _prep_inputs(features, weights, m, N):
    # wt[f, e*64+c] = weights[e, c, f]
    wt = np.ascontiguousarray(
        weights.astype(np.float32).transpose(2, 0, 1).reshape(F, E * C)
    ).astype(np.float16)
    cent = (m.astype(np.float32) / N.astype(np.float32)[None, :]).reshape(-1)  # [ec]
    negc = np.ascontiguousarray(-cent.reshape(MCH, 128).T)  # [128, MCH]
    feats = np.ascontiguousarray(
        features.astype(np.float16).reshape(B, F, PIX)
    )
    return [{"feat": feats[i], "wt": wt, "negc": negc} for i in range(B)]


def run_spmd(features, weights, m, N, trace=False):
    in_maps = _prep_inputs(features, weights, m, N)
    res = run_bass_kernel_spmd(_get_nc(), in_maps, list(range(B)), trace=trace)
    out = np.stack([res.results[i]["out"] for i in range(B)])  # [B, C, PIX]
    return out.reshape(B, C, H, W).astype(np.float32), res


def kernel(features, weights, m, N):
    out, _ = run_spmd(features, weights, m, N, trace=False)
    return out
